# revision 12
# baseline (speedup 1.0000x reference)
"""Bass/Trainium2 kernel for nn_Attention_5909874999334.

Multi-head attention (B=2, N=2048, DIM=1024, H=16, DH=64) on 8 NeuronCores:
data-parallel over batch x tensor-parallel over heads (4 heads/core).
Each core computes a partial (N, DIM) output through its row-slice of Wout;
the host sums the 4 partials per batch (the "all-reduce after to_out").

v3 design (vs the 292us baseline):
  - same attention engine split as baseline (ACT exps straight from PSUM,
    DVE does the bf16 mask-multiply) -- measured optimal: DVE PSUM-f32 reads
    run at ~1.3ns/elem vs 0.66 for SBUF bf16, ACT is ~1.05ns/elem anywhere.
  - q/k projection c-chunked against a chunked xt DMA so attention starts
    ~21us in (vs 55us); v-proj / qk(hp=1) / out-proj matmuls interleaved as
    filler inside the ACT-bound attention segments; no PE-idle tail.
  - out-projection head-pairs packed on K=128 (odd head rows placed at
    partitions 64-127 via a SBUF->SBUF DMA shift), halving its matmuls.
  - attention output evacuated PSUM->SBUF at segment end so the two PSUM
    accumulator banks recycle after ~2 jt of the next segment.
"""

import sys

sys.path.insert(0, "/opt/trn_rl_repo")

import numpy as np
import ml_dtypes

import concourse.bass as bass
from concourse import bacc
import concourse.tile as tile
from concourse import mybir
from concourse.bass_utils import run_bass_kernel_spmd

F32 = mybir.dt.float32
BF16 = mybir.dt.bfloat16
FP16 = mybir.dt.float16

B, N, DIM, H, DH = 2, 2048, 1024, 16, 64
INNER = H * DH          # 1024
HC = 4                  # heads per core
E = HC * DH             # 256 inner cols per core
NT = N // 128           # 16 token tiles
CT = DIM // 128         # 8 contraction chunks
SCALE = DH ** -0.5      # 0.125

IB = 512                # i-block
NIB = N // IB           # 4
WARM = 120              # PE warmup matmuls (N=64, ~6.4us)


def build_nc():
    nc = bacc.Bacc()
    xt_ext = nc.declare_dram_parameter("xt", [DIM, N], BF16, isOutput=False)
    wq_ext = nc.declare_dram_parameter("wq", [DIM, E], BF16, isOutput=False)
    wk_ext = nc.declare_dram_parameter("wk", [DIM, E], BF16, isOutput=False)
    wv_ext = nc.declare_dram_parameter("wv", [DIM, E], BF16, isOutput=False)
    wo_ext = nc.declare_dram_parameter("wo2", [128, 2, DIM], BF16, isOutput=False)
    vld_ext = nc.declare_dram_parameter("validT", [N, N], BF16, isOutput=False)
    out_ext = nc.declare_dram_parameter("out", [N, DIM], BF16, isOutput=True)

    Exp = mybir.ActivationFunctionType.Exp

    with tile.TileContext(nc) as tc:
        with (
            tc.tile_pool(name="persist", bufs=1) as pp,
            tc.tile_pool(name="ptp", bufs=3) as ptp,
            tc.tile_pool(name="pmp", bufs=4) as pmp,
            tc.tile_pool(name="vtp", bufs=10) as vtp,
            tc.tile_pool(name="nrm", bufs=4) as nrm,
            tc.tile_pool(name="ost", bufs=4) as ost,
            tc.tile_pool(name="psS", bufs=2, space="PSUM") as psS,
            tc.tile_pool(name="psO", bufs=2, space="PSUM") as psO,
            tc.tile_pool(name="psT", bufs=2, space="PSUM") as psT,
        ):
            # ---- persistent SBUF ----
            xt = pp.tile([128, CT, N], BF16, tag="xt")
            wq = pp.tile([128, CT, E], BF16, tag="wq")
            wk = pp.tile([128, CT, E], BF16, tag="wk")
            wv = pp.tile([128, CT, E], BF16, tag="wv")
            wo = pp.tile([128, 2, DIM], BF16, tag="wo")
            qT = [pp.tile([128, N], BF16, tag=f"qT{i}", name=f"qT{i}") for i in range(2)]
            kT = [pp.tile([128, N], BF16, tag=f"kT{i}", name=f"kT{i}") for i in range(2)]
            vaug = pp.tile([128, NT, HC, DH + 1], BF16, tag="vaug")
            outT2 = pp.tile([128, 2, N], BF16, tag="outT2")
            wrm = pp.tile([64, 64], BF16, tag="wrm")
            ones16 = pp.tile([1, 64], FP16, tag="ones16")
            dumi = pp.tile([1, 8], F32, tag="dumi")
            dumo = pp.tile([1, 8], BF16, tag="dumo")

            # ---- input DMAs (chunked xt so projections start early) ----
            nc.sync.dma_start(out=wq, in_=wq_ext.rearrange("(c p) e -> p c e", p=128))
            nc.sync.dma_start(out=wk, in_=wk_ext.rearrange("(c p) e -> p c e", p=128))
            for c in range(CT):
                nc.sync.dma_start(out=xt[:, c, :],
                                  in_=xt_ext[c * 128:(c + 1) * 128, :])
            nc.sync.dma_start(out=wv, in_=wv_ext.rearrange("(c p) e -> p c e", p=128))
            nc.sync.dma_start(out=wo, in_=wo_ext[:, :, :])
            nc.vector.memset(vaug[:, :, :, DH:DH + 1], 1.0)
            nc.vector.memset(wrm, 0.0)
            nc.vector.memset(ones16, 1.0)
            nc.vector.memset(dumi, 0.0)
            # preload the exp table set while DMAs land
            nc.scalar.activation(out=dumo, in_=dumi, func=Exp)

            # PE warmup (keeps HAM at 8/8 and covers input DMA latency)
            wps = psT.tile([128, IB], F32, tag="wps", name="warmps")
            for _ in range(WARM):
                nc.tensor.matmul(wps[0:64, 0:64], wrm, wrm, start=True, stop=True)

            # ---- qk0 projection (hp=0), c-chunk paced ----
            def qk_evac(dst, src):
                nc.vector.tensor_copy(out=dst, in_=src)

            accA = psS.tile([128, 2, IB], F32, tag="st", name="qk0a")
            accB = psS.tile([128, 2, IB], F32, tag="st", name="qk0b")
            accq1 = psT.tile([128, IB], F32, tag="wps", name="qk0q1")
            for c in range(CT):
                f, l = (c == 0), (c == CT - 1)
                nc.tensor.matmul(accA[:, 0, :], wq[:, c, 0:128], xt[:, c, 0:512], start=f, stop=l)
                nc.tensor.matmul(accA[:, 1, :], wk[:, c, 0:128], xt[:, c, 0:512], start=f, stop=l)
                nc.tensor.matmul(accB[:, 0, :], wk[:, c, 0:128], xt[:, c, 512:1024], start=f, stop=l)
                nc.tensor.matmul(accB[:, 1, :], wk[:, c, 0:128], xt[:, c, 1024:1536], start=f, stop=l)
                nc.tensor.matmul(accq1, wq[:, c, 0:128], xt[:, c, 512:1024], start=f, stop=l)
            qk_evac(qT[0][:, 0:512], accA[:, 0, :])
            qk_evac(kT[0][:, 0:512], accA[:, 1, :])
            qk_evac(kT[0][:, 512:1024], accB[:, 0, :])
            qk_evac(kT[0][:, 1024:1536], accB[:, 1, :])
            qk_evac(qT[0][:, 512:1024], accq1)

            # generic c-chunked projection block -> 4 chunk items + 1 evac item
            def make_proj_block(dst_col, w_sb, wlo, tok, bname):
                state = {}
                tsl = slice(tok * 512, (tok + 1) * 512)

                def chunk(cs):
                    if cs == 0:
                        state["a"] = psT.tile([128, IB], F32, tag="wps",
                                              name=bname)
                    for c in (cs * 2, cs * 2 + 1):
                        nc.tensor.matmul(state["a"], w_sb[:, c, wlo:wlo + 128],
                                         xt[:, c, tsl],
                                         start=(c == 0), stop=(c == CT - 1))

                def evac():
                    qk_evac(dst_col[:, tsl], state["a"])

                return [lambda cs=cs: chunk(cs) for cs in range(4)] + [evac]

            # ---- v projection tile ----
            def emit_vp(jt):
                vp = psT.tile([128, IB], F32, tag="wps", name=f"vp{jt}")
                for c in range(CT):
                    nc.tensor.matmul(vp[:, 0:E], xt[:, c, jt * 128:(jt + 1) * 128],
                                     wv[:, c, :], start=(c == 0), stop=(c == CT - 1))
                nc.vector.tensor_copy(
                    out=vaug[:, jt, :, 0:DH],
                    in_=vp[:, 0:E].rearrange("p (h d) -> p h d", h=HC))

            # ---- out-projection items (one psT tile each) ----
            def fp_item(ib, it, fh, ot_state):
                itg = ib * 4 + it
                if fh == 0:
                    ot_state[itg] = ost.tile([128, 2, IB], BF16, tag="ot",
                                             name=f"ot{itg}")
                ot = ot_state[itg]
                fpt = psT.tile([128, IB], F32, tag="wps", name=f"fp{itg}_{fh}")
                for g in range(2):
                    nc.tensor.matmul(
                        fpt, outT2[:, g, itg * 128:(itg + 1) * 128],
                        wo[:, g, fh * 512:(fh + 1) * 512],
                        start=(g == 0), stop=(g == 1))
                nc.vector.tensor_copy(out=ot[:, fh, :], in_=fpt)
                if fh == 1:
                    nc.sync.dma_start(
                        out=out_ext[itg * 128:(itg + 1) * 128, :], in_=ot)

            def make_fp_items(ib):
                ot_state = {}
                return [lambda ib=ib, it=it, fh=fh: fp_item(ib, it, fh, ot_state)
                        for it in range(4) for fh in range(2)]

            # ---- segment schedule ----
            SEGS = [(0, 0), (1, 0), (2, 0), (3, 0), (0, 1), (1, 1), (2, 1), (3, 1)]
            projq = (make_proj_block(qT[0], wq, 0, 2, "pb_q0_2")
                     + make_proj_block(kT[1], wk, 128, 0, "pb_k1_0")
                     + make_proj_block(kT[1], wk, 128, 1, "pb_k1_1")
                     + make_proj_block(kT[1], wk, 128, 2, "pb_k1_2")
                     + make_proj_block(qT[0], wq, 0, 3, "pb_q0_3")
                     + make_proj_block(kT[1], wk, 128, 3, "pb_k1_3")
                     + make_proj_block(qT[1], wq, 128, 0, "pb_q1_0")
                     + make_proj_block(qT[1], wq, 128, 1, "pb_q1_1")
                     + make_proj_block(qT[1], wq, 128, 2, "pb_q1_2")
                     + make_proj_block(qT[1], wq, 128, 3, "pb_q1_3"))
            fillers = {
                0: ([lambda jt=jt: emit_vp(jt) for jt in range(4)]
                    + make_proj_block(kT[0], wk, 0, 3, "pb_k0_3")
                    + [lambda jt=jt: emit_vp(jt) for jt in range(4, NT)], 2, 0),
                1: (projq, 1, 0),
                2: (projq, 1, 0),
                3: (projq, 1, 0),
                4: (projq, 1, 0),
                5: (make_fp_items(0), 2, 10),
                6: (make_fp_items(1), 2, 10),
                7: (make_fp_items(2), 2, 10),
            }

            def emit_av(oa, hp, jt, ptm):
                for hh in range(2):
                    nc.tensor.matmul(
                        oa[hh], vaug[:, jt, 2 * hp + hh, :], ptm[:, hh, :],
                        start=(jt == 0), stop=(jt == NT - 1))

            # pending_norm: state dict carrying prev-seg oa + SBUF copies
            def norm_evac(pn, hh):
                oa = pn["oa"]
                oab = nrm.tile([DH, IB], BF16, tag="oab", name=f"oab{pn['si']}_{hh}")
                nc.vector.tensor_copy(out=oab, in_=oa[hh][0:DH, :])
                dn = nrm.tile([1, IB], F32, tag="dn", name=f"dn{pn['si']}_{hh}")
                nc.vector.tensor_copy(out=dn, in_=oa[hh][DH:DH + 1, :])
                pn["oab"][hh] = oab
                pn["dn"][hh] = dn

            def norm_recip(pn, hh, fast=False):
                rc = nrm.tile([1, IB], F32, tag="rc", name=f"rc{pn['si']}_{hh}")
                nc.vector.reciprocal_approx_fast(out=rc, in_=pn["dn"][hh])
                if fast:
                    rcb = nrm.tile([1, IB], FP16, tag="rcb", name=f"rcb{pn['si']}_{hh}")
                    nc.vector.tensor_copy(out=rcb, in_=rc)
                    rpp = psT.tile([128, IB], F32, tag="wps", name=f"rpp{pn['si']}_{hh}")
                    nc.tensor.matmul(rpp[0:64, :], ones16, rcb, start=True, stop=True)
                    pn["rp"][hh] = rpp[0:64, :]
                else:
                    rp = nrm.tile([64, IB], F32, tag="rp", name=f"rp{pn['si']}_{hh}")
                    nc.gpsimd.partition_broadcast(rp, rc)
                    pn["rp"][hh] = rp

            def norm_mul(pn, hh):
                ib, hp = pn["ib"], pn["hp"]
                isl = slice(ib * IB, (ib + 1) * IB)
                if hh == 0:
                    nc.vector.tensor_mul(
                        out=outT2[0:64, hp, isl], in0=pn["oab"][0], in1=pn["rp"][0])
                else:
                    tmp = nrm.tile([64, IB], BF16, tag="tmp", name=f"tmp{pn['si']}")
                    nc.vector.tensor_mul(out=tmp, in0=pn["oab"][1], in1=pn["rp"][1])
                    nc.sync.dma_start(out=outT2[64:128, hp, isl], in_=tmp)

            pending_av = []       # [(oa, hp, jt, ptm)] from prev seg tail
            pending_norm = None

            for si, (ib, hp) in enumerate(SEGS):
                isl = slice(ib * IB, (ib + 1) * IB)
                oa = [psO.tile([DH + 1, IB], F32, tag="oa", name=f"oa{si}_{hh}")
                      for hh in range(2)]
                fill, rate, fstart = fillers.get(si, ([], 0, 0))
                if fill is not projq:
                    fill = list(fill)
                av_q = []  # (jt, ptm) awaiting emission (lag 2)
                for jt in range(NT):
                    # mask tile DMA
                    vt = vtp.tile([128, IB], BF16, tag="vt", name=f"vt{si}_{jt}")
                    nc.sync.dma_start(
                        out=vt, in_=vld_ext[jt * 128:(jt + 1) * 128, isl])
                    # prev-seg tail attn@V, then free its accumulators
                    if 1 <= jt <= 3 and pending_av:
                        p_oa, p_hp, p_jt, p_ptm = pending_av.pop(0)
                        emit_av(p_oa, p_hp, p_jt, p_ptm)
                        if not pending_av and pending_norm is not None:
                            norm_evac(pending_norm, 0)
                            norm_evac(pending_norm, 1)
                    if pending_norm is not None:
                        if jt == 4:
                            norm_recip(pending_norm, 0)
                            norm_recip(pending_norm, 1)
                        elif jt == 7:
                            norm_mul(pending_norm, 0)
                        elif jt == 8:
                            norm_mul(pending_norm, 1)
                            pending_norm = None
                    # filler matmuls (proj / out-proj)
                    if jt >= fstart:
                        for _ in range(rate):
                            if fill:
                                fill.pop(0)()
                    # sim pair (row-tiled T0/T8, concurrent)
                    st = psS.tile([128, 2, IB], F32, tag="st", name=f"st{si}_{jt}")
                    jsl = slice(jt * 128, (jt + 1) * 128)
                    nc.tensor.matmul(st[:, 0, :], kT[hp][0:64, jsl], qT[hp][0:64, isl],
                                     start=True, stop=True)
                    nc.tensor.matmul(st[:, 1, :], kT[hp][64:128, jsl], qT[hp][64:128, isl],
                                     start=True, stop=True)
                    # exp straight from PSUM (ACT), then bf16 mask-mul (DVE)
                    pt = ptp.tile([128, 2, IB], BF16, tag="pt", name=f"pt{si}_{jt}")
                    nc.scalar.activation(out=pt, in_=st, func=Exp, scale=SCALE)
                    ptm = pmp.tile([128, 2, IB], BF16, tag="ptm", name=f"ptm{si}_{jt}")
                    nc.vector.tensor_mul(
                        out=ptm, in0=pt,
                        in1=vt.unsqueeze(1).broadcast_to((128, 2, IB)))
                    av_q.append((jt, ptm))
                    # attn@V with 3-jt lag
                    if jt >= 3:
                        a_jt, a_ptm = av_q.pop(0)
                        emit_av(oa, hp, a_jt, a_ptm)
                pending_av = [(oa, hp, a_jt, a_ptm) for (a_jt, a_ptm) in av_q]
                pending_norm = {"oa": oa, "ib": ib, "hp": hp, "si": si,
                                "oab": {}, "dn": {}, "rp": {}}

            # ---- tail: last segment's final avs, norm, out-proj block 3 ----
            (oa0, hp0_, jt0_, ptm0) = pending_av[0]
            emit_av(oa0, hp0_, jt0_, ptm0)
            (oa1, hp1_, jt1_, ptm1) = pending_av[1]
            emit_av(oa1, hp1_, jt1_, ptm1)
            (oa2, hp2_, jt2_, ptm2) = pending_av[2]
            emit_av(oa2, hp2_, jt2_, ptm2)
            norm_evac(pending_norm, 0)
            norm_recip(pending_norm, 0, fast=True)
            norm_evac(pending_norm, 1)
            norm_recip(pending_norm, 1, fast=True)
            norm_mul(pending_norm, 0)
            norm_mul(pending_norm, 1)
            for item in make_fp_items(3):
                item()

    nc.finalize()
    return nc


_NC = None


def _get_nc():
    global _NC
    if _NC is None:
        _NC = build_nc()
    return _NC


def _install_trace_shim():
    """Provide antenv.axon_hooks for NTFF profiling under axon."""
    import types
    try:
        import antenv.axon_hooks  # noqa: F401
        return True
    except ImportError:
        pass
    try:
        from trn_agent_boot.trn_boot import _ntff_profile_via_ctypes
        hook = _ntff_profile_via_ctypes("/opt/axon/libaxon_pjrt.so")
    except Exception:
        return False
    if hook is None:
        return False
    mod = types.ModuleType("antenv.axon_hooks")
    mod.get_axon_ntff_profile_hook = lambda: hook
    sys.modules["antenv.axon_hooks"] = mod
    return True


def kernel(x, Wq, Wkv, Wout, attn_mask, key_padding_mask, _trace=False):
    x = np.asarray(x, dtype=np.float32)
    Wq = np.asarray(Wq, dtype=np.float32)
    Wkv = np.asarray(Wkv, dtype=np.float32)
    Wout = np.asarray(Wout, dtype=np.float32)
    attn_mask = np.asarray(attn_mask, dtype=bool)
    key_padding_mask = np.asarray(key_padding_mask, dtype=bool)

    nc = _get_nc()

    xT = [np.ascontiguousarray(x[b].T).astype(ml_dtypes.bfloat16) for b in range(B)]
    validT = []
    for b in range(B):
        v = ~(attn_mask.T | key_padding_mask[b][:, None])
        validT.append(v.astype(ml_dtypes.bfloat16))
    wq_s, wk_s, wv_s, wo_s = [], [], [], []
    for g in range(4):  # 4 head groups
        cols = slice(g * E, (g + 1) * E)
        wq_s.append(np.ascontiguousarray(Wq[:, cols]).astype(ml_dtypes.bfloat16))
        wk_s.append(np.ascontiguousarray(Wkv[:, cols]).astype(ml_dtypes.bfloat16))
        wv_s.append(np.ascontiguousarray(
            Wkv[:, INNER + g * E: INNER + (g + 1) * E]).astype(ml_dtypes.bfloat16))
        wl = Wout[cols, :].reshape(HC, DH, DIM)
        wo2 = np.empty((128, 2, DIM), dtype=np.float32)
        for gg in range(2):
            for a in range(2):
                wo2[a * 64:(a + 1) * 64, gg, :] = wl[2 * gg + a]
        wo_s.append(wo2.astype(ml_dtypes.bfloat16))

    in_maps = []
    for c in range(8):
        b, g = c // 4, c % 4
        in_maps.append({
            "xt": xT[b], "wq": wq_s[g], "wk": wk_s[g], "wv": wv_s[g],
            "wo2": wo_s[g], "validT": validT[b],
        })

    if _trace:
        _install_trace_shim()
    res = run_bass_kernel_spmd(nc, in_maps, core_ids=list(range(8)), trace=_trace)

    out = np.empty((B, N, DIM), dtype=np.float32)
    for b in range(B):
        acc = res.results[4 * b]["out"].astype(np.float32)
        for g in range(1, 4):
            acc = acc + res.results[4 * b + g]["out"].astype(np.float32)
        out[b] = acc
    if _trace:
        kernel.last_exec_time_ns = res.exec_time_ns
    return out


# revision 13
# speedup vs baseline: 1.0265x; 1.0265x over previous
"""Bass/Trainium2 kernel for nn_Attention_5909874999334.

Multi-head attention (B=2, N=2048, DIM=1024, H=16, DH=64) on 8 NeuronCores:
data-parallel over batch x tensor-parallel over heads (4 heads/core).
Each core computes a partial (N, DIM) output through its row-slice of Wout;
the host sums the 4 partials per batch (the "all-reduce after to_out").

v3 design (vs the 292us baseline):
  - same attention engine split as baseline (ACT exps straight from PSUM,
    DVE does the bf16 mask-multiply) -- measured optimal: DVE PSUM-f32 reads
    run at ~1.3ns/elem vs 0.66 for SBUF bf16, ACT is ~1.05ns/elem anywhere.
  - q/k projection c-chunked against a chunked xt DMA so attention starts
    ~21us in (vs 55us); v-proj / qk(hp=1) / out-proj matmuls interleaved as
    filler inside the ACT-bound attention segments; no PE-idle tail.
  - out-projection head-pairs packed on K=128 (odd head rows placed at
    partitions 64-127 via a SBUF->SBUF DMA shift), halving its matmuls.
  - attention output evacuated PSUM->SBUF at segment end so the two PSUM
    accumulator banks recycle after ~2 jt of the next segment.
"""

import sys

sys.path.insert(0, "/opt/trn_rl_repo")

import numpy as np
import ml_dtypes

import concourse.bass as bass
from concourse import bacc
import concourse.tile as tile
from concourse import mybir
from concourse.bass_utils import run_bass_kernel_spmd

F32 = mybir.dt.float32
BF16 = mybir.dt.bfloat16
FP16 = mybir.dt.float16

B, N, DIM, H, DH = 2, 2048, 1024, 16, 64
INNER = H * DH          # 1024
HC = 4                  # heads per core
E = HC * DH             # 256 inner cols per core
NT = N // 128           # 16 token tiles
CT = DIM // 128         # 8 contraction chunks
SCALE = DH ** -0.5      # 0.125

IB = 512                # i-block
NIB = N // IB           # 4
WARM = 120              # PE warmup matmuls (N=64, ~6.4us)


def build_nc():
    nc = bacc.Bacc()
    xt_ext = nc.declare_dram_parameter("xt", [DIM, N], BF16, isOutput=False)
    wq_ext = nc.declare_dram_parameter("wq", [DIM, E], BF16, isOutput=False)
    wk_ext = nc.declare_dram_parameter("wk", [DIM, E], BF16, isOutput=False)
    wv_ext = nc.declare_dram_parameter("wv", [DIM, E], BF16, isOutput=False)
    wo_ext = nc.declare_dram_parameter("wo2", [128, 2, DIM], BF16, isOutput=False)
    vld_ext = nc.declare_dram_parameter("validT", [N, N], BF16, isOutput=False)
    out_ext = nc.declare_dram_parameter("out", [N, DIM], BF16, isOutput=True)

    Exp = mybir.ActivationFunctionType.Exp

    with tile.TileContext(nc) as tc:
        with (
            tc.tile_pool(name="persist", bufs=1) as pp,
            tc.tile_pool(name="ptp", bufs=3) as ptp,
            tc.tile_pool(name="pmp", bufs=4) as pmp,
            tc.tile_pool(name="vtp", bufs=10) as vtp,
            tc.tile_pool(name="nrm", bufs=4) as nrm,
            tc.tile_pool(name="ost", bufs=4) as ost,
            tc.tile_pool(name="psS", bufs=2, space="PSUM") as psS,
            tc.tile_pool(name="psO", bufs=2, space="PSUM") as psO,
            tc.tile_pool(name="psT", bufs=2, space="PSUM") as psT,
        ):
            # ---- persistent SBUF ----
            xt = pp.tile([128, CT, N], BF16, tag="xt")
            wq = pp.tile([128, CT, E], BF16, tag="wq")
            wk = pp.tile([128, CT, E], BF16, tag="wk")
            wv = pp.tile([128, CT, E], BF16, tag="wv")
            wo = pp.tile([128, 2, DIM], BF16, tag="wo")
            qT = [pp.tile([128, N], BF16, tag=f"qT{i}", name=f"qT{i}") for i in range(2)]
            kT = [pp.tile([128, N], BF16, tag=f"kT{i}", name=f"kT{i}") for i in range(2)]
            vaug = pp.tile([128, NT, HC, DH + 1], BF16, tag="vaug")
            outT2 = pp.tile([128, 2, N], BF16, tag="outT2")
            wrm = pp.tile([64, 64], BF16, tag="wrm")
            ones16 = pp.tile([1, 64], FP16, tag="ones16")
            dumi = pp.tile([1, 8], F32, tag="dumi")
            dumo = pp.tile([1, 8], BF16, tag="dumo")

            # ---- input DMAs (chunked xt so projections start early) ----
            nc.sync.dma_start(out=wq, in_=wq_ext.rearrange("(c p) e -> p c e", p=128))
            nc.sync.dma_start(out=wk, in_=wk_ext.rearrange("(c p) e -> p c e", p=128))
            for c in range(CT):
                nc.sync.dma_start(out=xt[:, c, :],
                                  in_=xt_ext[c * 128:(c + 1) * 128, :])
            nc.sync.dma_start(out=wv, in_=wv_ext.rearrange("(c p) e -> p c e", p=128))
            nc.sync.dma_start(out=wo, in_=wo_ext[:, :, :])
            nc.vector.memset(vaug[:, :, :, DH:DH + 1], 1.0)
            nc.vector.memset(wrm, 0.0)
            nc.vector.memset(ones16, 1.0)
            nc.vector.memset(dumi, 0.0)
            # preload the exp table set while DMAs land
            nc.scalar.activation(out=dumo, in_=dumi, func=Exp)

            # PE warmup (keeps HAM at 8/8 and covers input DMA latency)
            wps = psT.tile([128, IB], F32, tag="wps", name="warmps")
            for _ in range(WARM):
                nc.tensor.matmul(wps[0:64, 0:64], wrm, wrm, start=True, stop=True)

            # ---- qk0 projection (hp=0), c-chunk paced ----
            def qk_evac(dst, src):
                nc.vector.tensor_copy(out=dst, in_=src)

            accA = psS.tile([128, 2, IB], F32, tag="st", name="qk0a")
            accB = psS.tile([128, 2, IB], F32, tag="st", name="qk0b")
            accq1 = psT.tile([128, IB], F32, tag="wps", name="qk0q1")
            for c in range(CT):
                f, l = (c == 0), (c == CT - 1)
                nc.tensor.matmul(accA[:, 0, :], wq[:, c, 0:128], xt[:, c, 0:512], start=f, stop=l)
                nc.tensor.matmul(accA[:, 1, :], wk[:, c, 0:128], xt[:, c, 0:512], start=f, stop=l)
                nc.tensor.matmul(accB[:, 0, :], wk[:, c, 0:128], xt[:, c, 512:1024], start=f, stop=l)
                nc.tensor.matmul(accB[:, 1, :], wk[:, c, 0:128], xt[:, c, 1024:1536], start=f, stop=l)
                nc.tensor.matmul(accq1, wq[:, c, 0:128], xt[:, c, 512:1024], start=f, stop=l)
            qk_evac(qT[0][:, 0:512], accA[:, 0, :])
            qk_evac(kT[0][:, 0:512], accA[:, 1, :])
            qk_evac(kT[0][:, 512:1024], accB[:, 0, :])
            qk_evac(kT[0][:, 1024:1536], accB[:, 1, :])
            qk_evac(qT[0][:, 512:1024], accq1)

            # generic c-chunked projection block -> 4 chunk items + 1 evac item
            def make_proj_block(dst_col, w_sb, wlo, tok, bname):
                state = {}
                tsl = slice(tok * 512, (tok + 1) * 512)

                def chunk(cs):
                    if cs == 0:
                        state["a"] = psT.tile([128, IB], F32, tag="wps",
                                              name=bname)
                    for c in (cs * 2, cs * 2 + 1):
                        nc.tensor.matmul(state["a"], w_sb[:, c, wlo:wlo + 128],
                                         xt[:, c, tsl],
                                         start=(c == 0), stop=(c == CT - 1))

                def evac():
                    qk_evac(dst_col[:, tsl], state["a"])

                return [lambda cs=cs: chunk(cs) for cs in range(4)] + [evac]

            # ---- v projection tile ----
            def emit_vp(jt):
                vp = psT.tile([128, IB], F32, tag="wps", name=f"vp{jt}")
                for c in range(CT):
                    nc.tensor.matmul(vp[:, 0:E], xt[:, c, jt * 128:(jt + 1) * 128],
                                     wv[:, c, :], start=(c == 0), stop=(c == CT - 1))
                nc.vector.tensor_copy(
                    out=vaug[:, jt, :, 0:DH],
                    in_=vp[:, 0:E].rearrange("p (h d) -> p h d", h=HC))

            # ---- out-projection items (one psT tile each) ----
            def fp_item(ib, it, fh, ot_state):
                itg = ib * 4 + it
                if fh == 0:
                    ot_state[itg] = ost.tile([128, 2, IB], BF16, tag="ot",
                                             name=f"ot{itg}")
                ot = ot_state[itg]
                fpt = psT.tile([128, IB], F32, tag="wps", name=f"fp{itg}_{fh}")
                for g in range(2):
                    nc.tensor.matmul(
                        fpt, outT2[:, g, itg * 128:(itg + 1) * 128],
                        wo[:, g, fh * 512:(fh + 1) * 512],
                        start=(g == 0), stop=(g == 1))
                nc.vector.tensor_copy(out=ot[:, fh, :], in_=fpt)
                if fh == 1:
                    nc.sync.dma_start(
                        out=out_ext[itg * 128:(itg + 1) * 128, :], in_=ot)

            def make_fp_items(ib):
                ot_state = {}
                return [lambda ib=ib, it=it, fh=fh: fp_item(ib, it, fh, ot_state)
                        for it in range(4) for fh in range(2)]

            # ---- segment schedule ----
            SEGS = [(0, 0), (1, 0), (2, 0), (3, 0), (0, 1), (1, 1), (2, 1), (3, 1)]
            projq = (make_proj_block(qT[0], wq, 0, 2, "pb_q0_2")
                     + make_proj_block(kT[1], wk, 128, 0, "pb_k1_0")
                     + make_proj_block(kT[1], wk, 128, 1, "pb_k1_1")
                     + make_proj_block(kT[1], wk, 128, 2, "pb_k1_2")
                     + make_proj_block(qT[0], wq, 0, 3, "pb_q0_3")
                     + make_proj_block(kT[1], wk, 128, 3, "pb_k1_3")
                     + make_proj_block(qT[1], wq, 128, 0, "pb_q1_0")
                     + make_proj_block(qT[1], wq, 128, 1, "pb_q1_1")
                     + make_proj_block(qT[1], wq, 128, 2, "pb_q1_2")
                     + make_proj_block(qT[1], wq, 128, 3, "pb_q1_3"))
            fillers = {
                0: ([lambda jt=jt: emit_vp(jt) for jt in range(4)]
                    + make_proj_block(kT[0], wk, 0, 3, "pb_k0_3")
                    + [lambda jt=jt: emit_vp(jt) for jt in range(4, NT)], 2, 0),
                1: (projq, 1, 0),
                2: (projq, 1, 0),
                3: (projq, 1, 0),
                4: (projq, 1, 0),
                5: (make_fp_items(0), 2, 10),
                6: (make_fp_items(1), 2, 10),
                7: (make_fp_items(2), 2, 10),
            }

            def emit_av(oa, hp, jt, ptm):
                for hh in range(2):
                    nc.tensor.matmul(
                        oa[hh], vaug[:, jt, 2 * hp + hh, :], ptm[:, hh, :],
                        start=(jt == 0), stop=(jt == NT - 1))

            # pending_norm: state dict carrying prev-seg oa + SBUF copies
            def norm_evac(pn, hh):
                oa = pn["oa"]
                oab = nrm.tile([DH, IB], BF16, tag="oab", name=f"oab{pn['si']}_{hh}")
                nc.vector.tensor_copy(out=oab, in_=oa[hh][0:DH, :])
                dn = nrm.tile([1, IB], F32, tag="dn", name=f"dn{pn['si']}_{hh}")
                nc.vector.tensor_copy(out=dn, in_=oa[hh][DH:DH + 1, :])
                pn["oab"][hh] = oab
                pn["dn"][hh] = dn

            def norm_recip(pn, hh, fast=False):
                rc = nrm.tile([1, IB], F32, tag="rc", name=f"rc{pn['si']}_{hh}")
                nc.vector.reciprocal_approx_fast(out=rc, in_=pn["dn"][hh])
                if fast:
                    rcb = nrm.tile([1, IB], FP16, tag="rcb", name=f"rcb{pn['si']}_{hh}")
                    nc.vector.tensor_copy(out=rcb, in_=rc)
                    rpp = psT.tile([128, IB], F32, tag="wps", name=f"rpp{pn['si']}_{hh}")
                    nc.tensor.matmul(rpp[0:64, :], ones16, rcb, start=True, stop=True)
                    pn["rp"][hh] = rpp[0:64, :]
                else:
                    rp = nrm.tile([64, IB], F32, tag="rp", name=f"rp{pn['si']}_{hh}")
                    nc.gpsimd.partition_broadcast(rp, rc)
                    pn["rp"][hh] = rp

            def norm_mul(pn, hh):
                ib, hp = pn["ib"], pn["hp"]
                isl = slice(ib * IB, (ib + 1) * IB)
                if hh == 0:
                    nc.vector.tensor_mul(
                        out=outT2[0:64, hp, isl], in0=pn["oab"][0], in1=pn["rp"][0])
                else:
                    tmp = nrm.tile([64, IB], BF16, tag="tmp", name=f"tmp{pn['si']}")
                    nc.vector.tensor_mul(out=tmp, in0=pn["oab"][1], in1=pn["rp"][1])
                    nc.sync.dma_start(out=outT2[64:128, hp, isl], in_=tmp)

            pending_av = []       # [(oa, hp, jt, ptm)] from prev seg tail
            pending_norm = None

            for si, (ib, hp) in enumerate(SEGS):
                isl = slice(ib * IB, (ib + 1) * IB)
                oa = [psO.tile([DH + 1, IB], F32, tag="oa", name=f"oa{si}_{hh}")
                      for hh in range(2)]
                fill, rate, fstart = fillers.get(si, ([], 0, 0))
                if fill is not projq:
                    fill = list(fill)
                av_q = []  # (jt, ptm) awaiting emission (lag 2)
                for jt in range(NT):
                    # mask tile DMA
                    vt = vtp.tile([128, IB], BF16, tag="vt", name=f"vt{si}_{jt}")
                    nc.sync.dma_start(
                        out=vt, in_=vld_ext[jt * 128:(jt + 1) * 128, isl])
                    # prev-seg tail attn@V, then free its accumulators
                    if 1 <= jt <= 3 and pending_av:
                        p_oa, p_hp, p_jt, p_ptm = pending_av.pop(0)
                        emit_av(p_oa, p_hp, p_jt, p_ptm)
                        if not pending_av and pending_norm is not None:
                            norm_evac(pending_norm, 0)
                            norm_evac(pending_norm, 1)
                    if pending_norm is not None:
                        if jt == 4:
                            norm_recip(pending_norm, 0)
                            norm_recip(pending_norm, 1)
                        elif jt == 7:
                            norm_mul(pending_norm, 0)
                        elif jt == 8:
                            norm_mul(pending_norm, 1)
                            pending_norm = None
                    # filler matmuls (proj / out-proj)
                    if jt >= fstart:
                        for _ in range(rate):
                            if fill:
                                fill.pop(0)()
                    # sim pair (row-tiled T0/T8, concurrent)
                    st = psS.tile([128, 2, IB], F32, tag="st", name=f"st{si}_{jt}")
                    jsl = slice(jt * 128, (jt + 1) * 128)
                    nc.tensor.matmul(st[:, 0, :], kT[hp][0:64, jsl], qT[hp][0:64, isl],
                                     start=True, stop=True)
                    nc.tensor.matmul(st[:, 1, :], kT[hp][64:128, jsl], qT[hp][64:128, isl],
                                     start=True, stop=True)
                    # exp straight from PSUM (ACT), then bf16 mask-mul (DVE)
                    pt = ptp.tile([128, 2, IB], BF16, tag="pt", name=f"pt{si}_{jt}")
                    nc.scalar.activation(out=pt, in_=st, func=Exp, scale=SCALE)
                    ptm = pmp.tile([128, 2, IB], BF16, tag="ptm", name=f"ptm{si}_{jt}")
                    nc.vector.tensor_mul(
                        out=ptm, in0=pt,
                        in1=vt.unsqueeze(1).broadcast_to((128, 2, IB)))
                    av_q.append((jt, ptm))
                    # attn@V with 3-jt lag
                    if jt >= 3:
                        a_jt, a_ptm = av_q.pop(0)
                        emit_av(oa, hp, a_jt, a_ptm)
                pending_av = [(oa, hp, a_jt, a_ptm) for (a_jt, a_ptm) in av_q]
                pending_norm = {"oa": oa, "ib": ib, "hp": hp, "si": si,
                                "oab": {}, "dn": {}, "rp": {}}

            # ---- tail: last segment's final avs, norm, out-proj block 3 ----
            for (p_oa, p_hp, p_jt, p_ptm) in pending_av:
                emit_av(p_oa, p_hp, p_jt, p_ptm)
            # keep-alive matmuls so the out-proj block below stays at 2.4GHz
            wrm2 = psS.tile([128, 2, IB], F32, tag="st", name="warmtail")
            for _ in range(40):
                nc.tensor.matmul(wrm2[0:64, 0, 0:64], wrm, wrm, start=True, stop=True)
            # tail norm: ACT (idle now) does the PSUM copies, DVE the recips
            pn = pending_norm
            for hh in range(2):
                oab = nrm.tile([DH, IB], BF16, tag="oab", name=f"oabT_{hh}")
                nc.scalar.copy(out=oab, in_=pn["oa"][hh][0:DH, :])
                dn = nrm.tile([1, IB], F32, tag="dn", name=f"dnT_{hh}")
                nc.scalar.copy(out=dn, in_=pn["oa"][hh][DH:DH + 1, :])
                pn["oab"][hh] = oab
                pn["dn"][hh] = dn
                norm_recip(pn, hh, fast=True)
            norm_mul(pn, 0)
            norm_mul(pn, 1)
            for item in make_fp_items(3):
                item()

    nc.finalize()
    return nc


_NC = None


def _get_nc():
    global _NC
    if _NC is None:
        _NC = build_nc()
    return _NC


def _install_trace_shim():
    """Provide antenv.axon_hooks for NTFF profiling under axon."""
    import types
    try:
        import antenv.axon_hooks  # noqa: F401
        return True
    except ImportError:
        pass
    try:
        from trn_agent_boot.trn_boot import _ntff_profile_via_ctypes
        hook = _ntff_profile_via_ctypes("/opt/axon/libaxon_pjrt.so")
    except Exception:
        return False
    if hook is None:
        return False
    mod = types.ModuleType("antenv.axon_hooks")
    mod.get_axon_ntff_profile_hook = lambda: hook
    sys.modules["antenv.axon_hooks"] = mod
    return True


def kernel(x, Wq, Wkv, Wout, attn_mask, key_padding_mask, _trace=False):
    x = np.asarray(x, dtype=np.float32)
    Wq = np.asarray(Wq, dtype=np.float32)
    Wkv = np.asarray(Wkv, dtype=np.float32)
    Wout = np.asarray(Wout, dtype=np.float32)
    attn_mask = np.asarray(attn_mask, dtype=bool)
    key_padding_mask = np.asarray(key_padding_mask, dtype=bool)

    nc = _get_nc()

    xT = [np.ascontiguousarray(x[b].T).astype(ml_dtypes.bfloat16) for b in range(B)]
    validT = []
    for b in range(B):
        v = ~(attn_mask.T | key_padding_mask[b][:, None])
        validT.append(v.astype(ml_dtypes.bfloat16))
    wq_s, wk_s, wv_s, wo_s = [], [], [], []
    for g in range(4):  # 4 head groups
        cols = slice(g * E, (g + 1) * E)
        wq_s.append(np.ascontiguousarray(Wq[:, cols]).astype(ml_dtypes.bfloat16))
        wk_s.append(np.ascontiguousarray(Wkv[:, cols]).astype(ml_dtypes.bfloat16))
        wv_s.append(np.ascontiguousarray(
            Wkv[:, INNER + g * E: INNER + (g + 1) * E]).astype(ml_dtypes.bfloat16))
        wl = Wout[cols, :].reshape(HC, DH, DIM)
        wo2 = np.empty((128, 2, DIM), dtype=np.float32)
        for gg in range(2):
            for a in range(2):
                wo2[a * 64:(a + 1) * 64, gg, :] = wl[2 * gg + a]
        wo_s.append(wo2.astype(ml_dtypes.bfloat16))

    in_maps = []
    for c in range(8):
        b, g = c // 4, c % 4
        in_maps.append({
            "xt": xT[b], "wq": wq_s[g], "wk": wk_s[g], "wv": wv_s[g],
            "wo2": wo_s[g], "validT": validT[b],
        })

    if _trace:
        _install_trace_shim()
    res = run_bass_kernel_spmd(nc, in_maps, core_ids=list(range(8)), trace=_trace)

    out = np.empty((B, N, DIM), dtype=np.float32)
    for b in range(B):
        acc = res.results[4 * b]["out"].astype(np.float32)
        for g in range(1, 4):
            acc = acc + res.results[4 * b + g]["out"].astype(np.float32)
        out[b] = acc
    if _trace:
        kernel.last_exec_time_ns = res.exec_time_ns
    return out
